# revision 50
# baseline (speedup 1.0000x reference)
"""Trainium2 Bass kernel for nn_Attention_3513283248742.

Bilinear attention: scores = h @ W @ b^T, attn = softmax(scores, -1),
ctx = attn @ b.  Shapes: b [32,1024,1024], h [32,256,1024], W_b [1,1024,1024].

Sharding: data-parallel over batch B=32 across 8 NeuronCores (4 batches per
core); W replicated.  No collectives.

Per-core pipeline (per batch):
  hT   = transpose(h_i)                       TensorE transposes (f32r)
  hWT  = W^T @ h^T  (= (hW)^T)                lhsT = W as stored, rhs = hT
  bT   = transpose(b_i)                       64 TensorE transposes
  S    = hWT^T @ bT  (= scores [q,k])         lhsT = hWT, rhs = bT
  softmax over k (free axis): exact row max (DVE), exp+rowsum fused on ACT
  attnT = transpose(E)                        E = exp(scores - max), unnormalized
  ctx  = attnT^T @ b ( = E @ b )              lhsT = attnT, rhs = b as stored
  out  = ctx * (1/rowsum)                     fused into ACT copy epilogue

The entire PE stream (all matmuls + transposes) runs in float16: inputs are
cast f32->fp16 on DVE/ACT right after DMA.  A pure 16-bit stream keeps the
fast-weight-load path enabled (FWL is disabled whenever the previous matmul
was FP32-HI, so mixing f32r and fp16 matmuls paces ~15% slower).  PSUM
accumulation stays fp32; softmax max/sum and the output epilogue are fp32.
Rel err ~3e-3 vs the f32 reference (gate 2e-2).
"""

import numpy as np

import concourse.bass as bass
import concourse.mybir as mybir
import concourse.tile as tile
from concourse.bass_utils import run_bass_kernel_spmd
from concourse.vector_clock import ScopedClock

F32 = mybir.dt.float32
F32R = mybir.dt.float32r
F16 = mybir.dt.float16

N_CORES = 8
B, TB, TH, D = 32, 1024, 1024, 1024  # TB=passage len; TH set below
TH = 256
BPC = B // N_CORES  # batches per core = 4
P = 128
NDC = D // P   # 8 chunks of the D axis
NKC = TB // P  # 8 chunks of the k axis
NQ = TH // P   # 2 chunks of the q axis

_PATCHED = False
CLEAR_SEMS_ON_EXIT = True


def _patch_tile_drain(max_waits_per_inst: int = 1):
    """This walrus build rejects >1 sem wait on the SP Drain instruction that
    TileContext emits on exit; split the waits across preceding sync nops."""
    global _PATCHED
    if _PATCHED:
        return
    _PATCHED = True

    def _drain_and_barrier(self, tick_clock, wait_clock):
        nc = self.nc
        drain_inst = nc.sync.drain()
        wait_clock.add_sem_waits(
            drain_inst.ins, ScopedClock({None: tick_clock.global_clock})
        )
        si = drain_inst.ins.sync_info
        if si is not None and si.on_wait and len(si.on_wait) > max_waits_per_inst:
            waits = list(si.on_wait)
            bb = nc.cur_bb.bb
            assert bb.instructions[-1] is drain_inst.ins
            bb.instructions.pop()
            si.on_wait = waits[:max_waits_per_inst]
            rest = waits[max_waits_per_inst:]
            for i in range(0, len(rest), max_waits_per_inst):
                nop = nc.sync.nop(nofuse=True)
                chunk = rest[i : i + max_waits_per_inst]
                if nop.ins.sync_info is None:
                    nop.ins.sync_info = mybir.SyncInfo(on_wait=chunk, on_update=[])
                else:
                    nop.ins.sync_info.on_wait.extend(chunk)
            bb.instructions.append(drain_inst.ins)
        nc.all_engine_barrier()
        assert self.sems is not None
        popped = nc._tile_sem_poison_stack.pop()
        assert popped is self._sem_poison
        if CLEAR_SEMS_ON_EXIT:
            nc.clear_and_free_semaphores(list(self.sems.allocated().values()))
            nc.all_engine_barrier()
        else:
            nc._state.prepend_free_semaphores(
                [
                    s.num if hasattr(s, "num") else s
                    for s in self.sems.allocated().values()
                ]
            )

    tile.TileContext._drain_and_barrier = _drain_and_barrier


def _split_excess_waits(nc, max_waits: int = 1):
    """Walrus rejects instructions carrying more than `max_waits` sem waits.
    Hoist excess waits onto same-engine nops inserted just before."""
    for f in nc.m.functions:
        for bb in f.blocks:
            out = []
            for ins in list(bb.instructions):
                si = ins.sync_info
                if si is not None and si.on_wait and len(si.on_wait) > max_waits:
                    waits = list(si.on_wait)
                    si.on_wait = waits[:max_waits]
                    rest = waits[max_waits:]
                    for i in range(0, len(rest), max_waits):
                        nop = nc.engines[ins.engine].nop(nofuse=True)
                        cur_bb = nc.cur_bb.bb
                        assert cur_bb.instructions[-1] is nop.ins
                        cur_bb.instructions.pop()
                        nop.ins.sync_info = mybir.SyncInfo(
                            on_wait=rest[i : i + max_waits], on_update=[]
                        )
                        out.append(nop.ins)
                out.append(ins)
            bb.instructions[:] = out


def build_nc():
    _patch_tile_drain()
    nc = bass.Bass(trn_type="TRN2", target_bir_lowering=False, debug=False)
    b_ext = nc.declare_dram_parameter("b", [BPC, TB, D], F16, isOutput=False)
    h_ext = nc.declare_dram_parameter("h", [BPC, TH, D], F16, isOutput=False)
    w_ext = nc.declare_dram_parameter("w", [D, D], F16, isOutput=False)
    ident_ext = nc.declare_dram_parameter("ident", [P, P], F16, isOutput=False)
    out_ext = nc.declare_dram_parameter("out", [BPC, TH, D], F32, isOutput=True)

    with tile.TileContext(nc) as tc:
        with (
            tc.tile_pool(name="consts", bufs=1) as consts,
            tc.tile_pool(name="bpool", bufs=2) as bpool,
            tc.tile_pool(name="btpool", bufs=2) as btpool,
            tc.tile_pool(name="hpool", bufs=1) as hpool,
            tc.tile_pool(name="mid", bufs=2) as mid,
            tc.tile_pool(name="ctxpool", bufs=2) as ctxpool,
            tc.tile_pool(name="stats", bufs=2) as stats,
            tc.tile_pool(name="psbig", bufs=2, space="PSUM") as psbig,
            tc.tile_pool(name="pssm", bufs=4, space="PSUM") as pssm,
        ):
            # --- constants ---
            # Startup DMA order: ident + h0 first (PE starts h-transposes
            # ASAP), then W in chunks (hWT matmuls stream behind them), then
            # b0 in chunks (b-transposes stream behind those).
            ident16_t = consts.tile([P, P], F16)
            nc.sync.dma_start(ident16_t[:], ident_ext.ap())
            ident16 = ident16_t[:]
            w16_sb = consts.tile([P, NDC, D], F16)  # [din(part), j, dout]

            # --- per-batch emission helpers (closures over per-batch state) ---
            def emit_load_h(i):
                h16_sb = hpool.tile([P, NQ, D], F16, name=f"h16_{i}", tag="h16")
                for r in range(NQ):
                    nc.sync.dma_start(
                        h16_sb[:, r, :], h_ext[i, r * P : (r + 1) * P, :]
                    )
                return h16_sb

            def emit_load_b(i):
                b16_sb = bpool.tile([P, NKC, D], F16, name=f"b16_{i}", tag="b16")
                if i == 0:
                    for c in range(NKC):
                        nc.sync.dma_start(
                            b16_sb[:, c, :], b_ext[i, c * P : (c + 1) * P, :]
                        )
                else:
                    nc.sync.dma_start(
                        b16_sb[:], b_ext[i].rearrange("(c p) d -> p c d", p=P)
                    )
                return b16_sb

            def emit_hT(i, h16_sb):
                # hT[d, q] : fp16 transposes of h
                hT_sb = mid.tile([P, NDC, TH], F16, name=f"hT{i}", tag="hT")
                for jp in range(0, NDC, 4):
                    ps = pssm.tile([P, 1024], F16, name="ps16", tag="ps")
                    for dj in range(4):
                        j = jp + dj
                        for r in range(NQ):
                            nc.tensor.transpose(
                                ps[:, dj * 256 + r * P : dj * 256 + (r + 1) * P],
                                h16_sb[:, r, j * P : (j + 1) * P],
                                ident16,
                            )
                    nc.vector.tensor_copy(
                        hT_sb[:, jp : jp + 4, :].rearrange("p a b -> p (a b)"),
                        ps[:],
                    )
                return hT_sb

            def emit_bT(i, b16_sb):
                # bT[d, k] : fp16 transposes (k-chunk-major)
                bT_sb = btpool.tile([P, NDC, TB], F16, name=f"bT{i}", tag="bT")
                for c in range(NKC):
                    ps = pssm.tile([P, 1024], F16, name="ps16", tag="ps")
                    for j in range(NDC):
                        nc.tensor.transpose(
                            ps[:, j * P : (j + 1) * P],
                            b16_sb[:, c, j * P : (j + 1) * P],
                            ident16,
                        )
                    eng = nc.vector.tensor_copy if (c % 2 == 0) else nc.scalar.copy
                    eng(
                        bT_sb[:, :, c * P : (c + 1) * P],
                        ps[:].rearrange("p (a b) -> p a b", a=NDC),
                    )
                return bT_sb

            def emit_hWT(i, hT_sb):
                # hWT[dout, q] = W^T @ hT  (accumulate over din chunks)
                hWT_sb = mid.tile([P, NDC, TH], F16, name=f"hWT{i}", tag="hWT")
                for tp in range(0, NDC, 2):
                    ps = pssm.tile([P, 512], F32, name="ps", tag="ps")
                    for dt in range(2):
                        t = tp + dt
                        for j in range(NDC):
                            nc.tensor.matmul(
                                ps[:, dt * 256 : (dt + 1) * 256],
                                w16_sb[:, j, t * P : (t + 1) * P],
                                hT_sb[:, j, :],
                                start=(j == 0),
                                stop=(j == NDC - 1),
                            )
                    nc.scalar.copy(
                        hWT_sb[:, tp : tp + 2, :].rearrange("p a b -> p (a b)"),
                        ps[:],
                    )
                return hWT_sb

            # --- software pipeline over batches ---
            # Next batch's loads + hT transposes are emitted before ctx(0) so
            # the PE has independent work while DVE/ACT finish softmax/attnT
            # copies of the current batch.
            h16_sb = emit_load_h(0)
            for j in range(NDC):
                nc.sync.dma_start(w16_sb[:, j, :], w_ext[j * P : (j + 1) * P, :])
            b16_sb = emit_load_b(0)

            hT_sb = emit_hT(0, h16_sb)

            for i in range(BPC):
                hWT_sb = emit_hWT(i, hT_sb)
                bT_sb = emit_bT(i, b16_sb)

                E_sb = mid.tile([P, NQ, TB], F16, name=f"E{i}", tag="E")
                negmax = stats.tile([P, NQ, 1], F32, name="negmax", tag="negmax")
                S_sum = stats.tile([P, NQ, 1], F32, name="S_sum", tag="S")
                invS = stats.tile([P, NQ, 1], F32, name="invS", tag="invS")
                attnT = [
                    mid.tile([P, NKC, P], F16, name=f"attnT{i}_{r}", tag=f"attnT{r}")
                    for r in range(NQ)
                ]
                ctx_sb = ctxpool.tile([P, NQ, D], F32, name=f"ctx{i}", tag="ctx")
                ps_scores = [None] * NQ

                def scores_mm(r, hWT_sb=hWT_sb, bT_sb=bT_sb, ps_scores=ps_scores):
                    ps_s = psbig.tile([P, TB], F32, name="ps_s", tag="psb")
                    ps_scores[r] = ps_s
                    for kh in range(2):
                        for j in range(NDC):
                            nc.tensor.matmul(
                                ps_s[:, kh * 512 : (kh + 1) * 512],
                                hWT_sb[:, j, r * P : (r + 1) * P],
                                bT_sb[:, j, kh * 512 : (kh + 1) * 512],
                                start=(j == 0),
                                stop=(j == NDC - 1),
                            )

                def softmax_half(r, E_sb=E_sb, negmax=negmax, S_sum=S_sum,
                                 invS=invS, ps_scores=ps_scores):
                    ps_s = ps_scores[r]
                    nc.vector.tensor_reduce(
                        negmax[:, r, :],
                        ps_s[:],
                        axis=mybir.AxisListType.X,
                        op=mybir.AluOpType.max,
                        negate=True,
                    )
                    nc.scalar.activation(
                        E_sb[:, r, :],
                        ps_s[:],
                        mybir.ActivationFunctionType.Exp,
                        bias=negmax[:, r, :],
                        accum_out=S_sum[:, r, :],
                    )
                    nc.vector.reciprocal(invS[:, r, :], S_sum[:, r, :])

                def attnT_half(r, E_sb=E_sb, attnT=attnT):
                    ps = pssm.tile([P, 1024], F16, name="ps16", tag="ps")
                    for c in range(NKC):
                        nc.tensor.transpose(
                            ps[:, c * P : (c + 1) * P],
                            E_sb[:, r, c * P : (c + 1) * P],
                            ident16,
                        )
                    eng = nc.scalar.copy if r == 0 else nc.vector.tensor_copy
                    eng(
                        attnT[r][:].rearrange("p a b -> p (a b)"),
                        ps[:],
                    )

                def ctx_mm(r, attnT=attnT, b16_sb=b16_sb, ctx_sb=ctx_sb, invS=invS):
                    ps_c = psbig.tile([P, D], F32, name="ps_c", tag="psb")
                    for dh in range(2):
                        for c in range(NKC):
                            nc.tensor.matmul(
                                ps_c[:, dh * 512 : (dh + 1) * 512],
                                attnT[r][:, c, :],
                                b16_sb[:, c, dh * 512 : (dh + 1) * 512],
                                start=(c == 0),
                                stop=(c == NKC - 1),
                            )
                    nc.scalar.mul(ctx_sb[:, r, :], ps_c[:], invS[:, r, :])
                    nc.sync.dma_start(
                        out_ext[i, r * P : (r + 1) * P, :], ctx_sb[:, r, :]
                    )

                scores_mm(0)
                softmax_half(0)
                scores_mm(1)
                if i + 1 < BPC:
                    attnT_half(0)
                    softmax_half(1)
                    next_h16 = emit_load_h(i + 1)
                    next_b16 = emit_load_b(i + 1)
                    ctx_mm(0)
                    next_hT = emit_hT(i + 1, next_h16)
                    attnT_half(1)
                    ctx_mm(1)
                else:
                    # last batch: no next-batch PE filler exists, so issue
                    # softmax(1) ASAP (ACT/DVE are idle enough now) and let
                    # attnT(0)+ctx(0) cover its latency
                    softmax_half(1)
                    attnT_half(0)
                    ctx_mm(0)
                    attnT_half(1)
                    ctx_mm(1)

                if i + 1 < BPC:
                    h16_sb, b16_sb, hT_sb = next_h16, next_b16, next_hT
    _split_excess_waits(nc)
    return nc


_NC_CACHE = None


def _get_nc():
    global _NC_CACHE
    if _NC_CACHE is None:
        _NC_CACHE = build_nc()
    return _NC_CACHE


def run(b, h, W_b, trace=False):
    """Shard, execute on 8 cores, gather. Returns (ctx, BassKernelResults)."""
    assert b.shape == (B, TB, D) and h.shape == (B, TH, D)
    # inputs are consumed on-chip exclusively in fp16 -> convert on the host
    # (halves all input DMA traffic and removes every on-chip cast)
    W16 = np.ascontiguousarray(W_b[0].astype(np.float16))
    b16 = np.ascontiguousarray(b.astype(np.float16))
    h16 = np.ascontiguousarray(h.astype(np.float16))
    in_maps = []
    for c in range(N_CORES):
        sl = slice(c * BPC, (c + 1) * BPC)
        in_maps.append(
            {
                "b": b16[sl],
                "h": h16[sl],
                "w": W16,
                "ident": np.eye(P, dtype=np.float16),
            }
        )
    res = run_bass_kernel_spmd(
        _get_nc(), in_maps, core_ids=list(range(N_CORES)), trace=trace
    )
    out = np.concatenate([res.results[c]["out"] for c in range(N_CORES)], axis=0)
    return out.astype(np.float32), res


def kernel(b, h, W_b):
    out, _ = run(b, h, W_b, trace=False)
    return out


# revision 52
# speedup vs baseline: 1.0131x; 1.0131x over previous
"""Trainium2 Bass kernel for nn_Attention_3513283248742.

Bilinear attention: scores = h @ W @ b^T, attn = softmax(scores, -1),
ctx = attn @ b.  Shapes: b [32,1024,1024], h [32,256,1024], W_b [1,1024,1024].

Sharding: data-parallel over batch B=32 across 8 NeuronCores (4 batches per
core); W replicated.  No collectives.

Per-core pipeline (per batch):
  hT   = transpose(h_i)                       TensorE transposes (f32r)
  hWT  = W^T @ h^T  (= (hW)^T)                lhsT = W as stored, rhs = hT
  bT   = transpose(b_i)                       64 TensorE transposes
  S    = hWT^T @ bT  (= scores [q,k])         lhsT = hWT, rhs = bT
  softmax over k (free axis): exact row max (DVE), exp+rowsum fused on ACT
  attnT = transpose(E)                        E = exp(scores - max), unnormalized
  ctx  = attnT^T @ b ( = E @ b )              lhsT = attnT, rhs = b as stored
  out  = ctx * (1/rowsum)                     fused into ACT copy epilogue

The entire PE stream (all matmuls + transposes) runs in float16: inputs are
cast f32->fp16 on DVE/ACT right after DMA.  A pure 16-bit stream keeps the
fast-weight-load path enabled (FWL is disabled whenever the previous matmul
was FP32-HI, so mixing f32r and fp16 matmuls paces ~15% slower).  PSUM
accumulation stays fp32; softmax max/sum and the output epilogue are fp32.
Rel err ~3e-3 vs the f32 reference (gate 2e-2).
"""

import numpy as np

import concourse.bass as bass
import concourse.mybir as mybir
import concourse.tile as tile
from concourse.bass_utils import run_bass_kernel_spmd
from concourse.vector_clock import ScopedClock

F32 = mybir.dt.float32
F32R = mybir.dt.float32r
F16 = mybir.dt.float16

N_CORES = 8
B, TB, TH, D = 32, 1024, 1024, 1024  # TB=passage len; TH set below
TH = 256
BPC = B // N_CORES  # batches per core = 4
P = 128
NDC = D // P   # 8 chunks of the D axis
NKC = TB // P  # 8 chunks of the k axis
NQ = TH // P   # 2 chunks of the q axis

_PATCHED = False
CLEAR_SEMS_ON_EXIT = True


def _patch_tile_drain(max_waits_per_inst: int = 1):
    """This walrus build rejects >1 sem wait on the SP Drain instruction that
    TileContext emits on exit; split the waits across preceding sync nops."""
    global _PATCHED
    if _PATCHED:
        return
    _PATCHED = True

    def _drain_and_barrier(self, tick_clock, wait_clock):
        nc = self.nc
        drain_inst = nc.sync.drain()
        wait_clock.add_sem_waits(
            drain_inst.ins, ScopedClock({None: tick_clock.global_clock})
        )
        si = drain_inst.ins.sync_info
        if si is not None and si.on_wait and len(si.on_wait) > max_waits_per_inst:
            waits = list(si.on_wait)
            bb = nc.cur_bb.bb
            assert bb.instructions[-1] is drain_inst.ins
            bb.instructions.pop()
            si.on_wait = waits[:max_waits_per_inst]
            rest = waits[max_waits_per_inst:]
            for i in range(0, len(rest), max_waits_per_inst):
                nop = nc.sync.nop(nofuse=True)
                chunk = rest[i : i + max_waits_per_inst]
                if nop.ins.sync_info is None:
                    nop.ins.sync_info = mybir.SyncInfo(on_wait=chunk, on_update=[])
                else:
                    nop.ins.sync_info.on_wait.extend(chunk)
            bb.instructions.append(drain_inst.ins)
        nc.all_engine_barrier()
        assert self.sems is not None
        popped = nc._tile_sem_poison_stack.pop()
        assert popped is self._sem_poison
        if CLEAR_SEMS_ON_EXIT:
            nc.clear_and_free_semaphores(list(self.sems.allocated().values()))
            nc.all_engine_barrier()
        else:
            nc._state.prepend_free_semaphores(
                [
                    s.num if hasattr(s, "num") else s
                    for s in self.sems.allocated().values()
                ]
            )

    tile.TileContext._drain_and_barrier = _drain_and_barrier


def _split_excess_waits(nc, max_waits: int = 1):
    """Walrus rejects instructions carrying more than `max_waits` sem waits.
    Hoist excess waits onto same-engine nops inserted just before."""
    for f in nc.m.functions:
        for bb in f.blocks:
            out = []
            for ins in list(bb.instructions):
                si = ins.sync_info
                if si is not None and si.on_wait and len(si.on_wait) > max_waits:
                    waits = list(si.on_wait)
                    si.on_wait = waits[:max_waits]
                    rest = waits[max_waits:]
                    for i in range(0, len(rest), max_waits):
                        nop = nc.engines[ins.engine].nop(nofuse=True)
                        cur_bb = nc.cur_bb.bb
                        assert cur_bb.instructions[-1] is nop.ins
                        cur_bb.instructions.pop()
                        nop.ins.sync_info = mybir.SyncInfo(
                            on_wait=rest[i : i + max_waits], on_update=[]
                        )
                        out.append(nop.ins)
                out.append(ins)
            bb.instructions[:] = out


def build_nc():
    _patch_tile_drain()
    nc = bass.Bass(trn_type="TRN2", target_bir_lowering=False, debug=False)
    b_ext = nc.declare_dram_parameter("b", [BPC, TB, D], F16, isOutput=False)
    h_ext = nc.declare_dram_parameter("h", [BPC, TH, D], F16, isOutput=False)
    w_ext = nc.declare_dram_parameter("w", [D, D], F16, isOutput=False)
    ident_ext = nc.declare_dram_parameter("ident", [P, P], F16, isOutput=False)
    out_ext = nc.declare_dram_parameter("out", [BPC, TH, D], F32, isOutput=True)

    with tile.TileContext(nc) as tc:
        with (
            tc.tile_pool(name="consts", bufs=1) as consts,
            tc.tile_pool(name="bpool", bufs=2) as bpool,
            tc.tile_pool(name="btpool", bufs=2) as btpool,
            tc.tile_pool(name="hpool", bufs=1) as hpool,
            tc.tile_pool(name="mid", bufs=2) as mid,
            tc.tile_pool(name="ctxpool", bufs=2) as ctxpool,
            tc.tile_pool(name="stats", bufs=2) as stats,
            tc.tile_pool(name="psbig", bufs=2, space="PSUM") as psbig,
            tc.tile_pool(name="pssm", bufs=4, space="PSUM") as pssm,
        ):
            # --- constants ---
            # Startup DMA order: ident + h0 first (PE starts h-transposes
            # ASAP), then W in chunks (hWT matmuls stream behind them), then
            # b0 in chunks (b-transposes stream behind those).
            ident16_t = consts.tile([P, P], F16)
            nc.sync.dma_start(ident16_t[:], ident_ext.ap())
            ident16 = ident16_t[:]
            # HAM warmup: ~24 pipelined identity transposes round-robin across
            # 4 PSUM banks (different banks -> no WAW serialization) right at
            # t=0 so the PE clock-gate reaches 2.4GHz before the prefix work.
            warm = [
                pssm.tile([P, 1024], F16, name=f"warm{k}", tag="ps")
                for k in range(4)
            ]
            for wi in range(24):
                nc.tensor.transpose(
                    warm[wi % 4][:, (wi // 4 % 8) * P : ((wi // 4 % 8) + 1) * P],
                    ident16,
                    ident16,
                )
            w16_sb = consts.tile([P, NDC, D], F16)  # [din(part), j, dout]

            # --- per-batch emission helpers (closures over per-batch state) ---
            def emit_load_h(i):
                h16_sb = hpool.tile([P, NQ, D], F16, name=f"h16_{i}", tag="h16")
                for r in range(NQ):
                    nc.sync.dma_start(
                        h16_sb[:, r, :], h_ext[i, r * P : (r + 1) * P, :]
                    )
                return h16_sb

            def emit_load_b(i):
                b16_sb = bpool.tile([P, NKC, D], F16, name=f"b16_{i}", tag="b16")
                if i == 0:
                    for c in range(NKC):
                        nc.sync.dma_start(
                            b16_sb[:, c, :], b_ext[i, c * P : (c + 1) * P, :]
                        )
                else:
                    nc.sync.dma_start(
                        b16_sb[:], b_ext[i].rearrange("(c p) d -> p c d", p=P)
                    )
                return b16_sb

            def emit_hT(i, h16_sb):
                # hT[d, q] : fp16 transposes of h
                hT_sb = mid.tile([P, NDC, TH], F16, name=f"hT{i}", tag="hT")
                for jp in range(0, NDC, 4):
                    ps = pssm.tile([P, 1024], F16, name="ps16", tag="ps")
                    for dj in range(4):
                        j = jp + dj
                        for r in range(NQ):
                            nc.tensor.transpose(
                                ps[:, dj * 256 + r * P : dj * 256 + (r + 1) * P],
                                h16_sb[:, r, j * P : (j + 1) * P],
                                ident16,
                            )
                    nc.vector.tensor_copy(
                        hT_sb[:, jp : jp + 4, :].rearrange("p a b -> p (a b)"),
                        ps[:],
                    )
                return hT_sb

            def emit_bT(i, b16_sb):
                # bT[d, k] : fp16 transposes (k-chunk-major)
                bT_sb = btpool.tile([P, NDC, TB], F16, name=f"bT{i}", tag="bT")
                for c in range(NKC):
                    ps = pssm.tile([P, 1024], F16, name="ps16", tag="ps")
                    for j in range(NDC):
                        nc.tensor.transpose(
                            ps[:, j * P : (j + 1) * P],
                            b16_sb[:, c, j * P : (j + 1) * P],
                            ident16,
                        )
                    eng = nc.vector.tensor_copy if (c % 2 == 0) else nc.scalar.copy
                    eng(
                        bT_sb[:, :, c * P : (c + 1) * P],
                        ps[:].rearrange("p (a b) -> p a b", a=NDC),
                    )
                return bT_sb

            def emit_hWT(i, hT_sb):
                # hWT[dout, q] = W^T @ hT  (accumulate over din chunks)
                hWT_sb = mid.tile([P, NDC, TH], F16, name=f"hWT{i}", tag="hWT")
                for tp in range(0, NDC, 2):
                    ps = pssm.tile([P, 512], F32, name="ps", tag="ps")
                    for dt in range(2):
                        t = tp + dt
                        for j in range(NDC):
                            nc.tensor.matmul(
                                ps[:, dt * 256 : (dt + 1) * 256],
                                w16_sb[:, j, t * P : (t + 1) * P],
                                hT_sb[:, j, :],
                                start=(j == 0),
                                stop=(j == NDC - 1),
                            )
                    nc.scalar.copy(
                        hWT_sb[:, tp : tp + 2, :].rearrange("p a b -> p (a b)"),
                        ps[:],
                    )
                return hWT_sb

            # --- software pipeline over batches ---
            # Next batch's loads + hT transposes are emitted before ctx(0) so
            # the PE has independent work while DVE/ACT finish softmax/attnT
            # copies of the current batch.
            h16_sb = emit_load_h(0)
            for j in range(NDC):
                nc.sync.dma_start(w16_sb[:, j, :], w_ext[j * P : (j + 1) * P, :])
            b16_sb = emit_load_b(0)

            hT_sb = emit_hT(0, h16_sb)

            for i in range(BPC):
                hWT_sb = emit_hWT(i, hT_sb)
                bT_sb = emit_bT(i, b16_sb)

                E_sb = mid.tile([P, NQ, TB], F16, name=f"E{i}", tag="E")
                negmax = stats.tile([P, NQ, 1], F32, name="negmax", tag="negmax")
                S_sum = stats.tile([P, NQ, 1], F32, name="S_sum", tag="S")
                invS = stats.tile([P, NQ, 1], F32, name="invS", tag="invS")
                attnT = [
                    mid.tile([P, NKC, P], F16, name=f"attnT{i}_{r}", tag=f"attnT{r}")
                    for r in range(NQ)
                ]
                ctx_sb = ctxpool.tile([P, NQ, D], F32, name=f"ctx{i}", tag="ctx")
                ps_scores = [None] * NQ

                def scores_mm(r, hWT_sb=hWT_sb, bT_sb=bT_sb, ps_scores=ps_scores):
                    ps_s = psbig.tile([P, TB], F32, name="ps_s", tag="psb")
                    ps_scores[r] = ps_s
                    for kh in range(2):
                        for j in range(NDC):
                            nc.tensor.matmul(
                                ps_s[:, kh * 512 : (kh + 1) * 512],
                                hWT_sb[:, j, r * P : (r + 1) * P],
                                bT_sb[:, j, kh * 512 : (kh + 1) * 512],
                                start=(j == 0),
                                stop=(j == NDC - 1),
                            )

                def softmax_half(r, E_sb=E_sb, negmax=negmax, S_sum=S_sum,
                                 invS=invS, ps_scores=ps_scores):
                    ps_s = ps_scores[r]
                    nc.vector.tensor_reduce(
                        negmax[:, r, :],
                        ps_s[:],
                        axis=mybir.AxisListType.X,
                        op=mybir.AluOpType.max,
                        negate=True,
                    )
                    nc.scalar.activation(
                        E_sb[:, r, :],
                        ps_s[:],
                        mybir.ActivationFunctionType.Exp,
                        bias=negmax[:, r, :],
                        accum_out=S_sum[:, r, :],
                    )
                    nc.vector.reciprocal(invS[:, r, :], S_sum[:, r, :])

                def attnT_half(r, E_sb=E_sb, attnT=attnT):
                    ps = pssm.tile([P, 1024], F16, name="ps16", tag="ps")
                    for c in range(NKC):
                        nc.tensor.transpose(
                            ps[:, c * P : (c + 1) * P],
                            E_sb[:, r, c * P : (c + 1) * P],
                            ident16,
                        )
                    eng = nc.scalar.copy if r == 0 else nc.vector.tensor_copy
                    eng(
                        attnT[r][:].rearrange("p a b -> p (a b)"),
                        ps[:],
                    )

                def ctx_mm(r, attnT=attnT, b16_sb=b16_sb, ctx_sb=ctx_sb, invS=invS):
                    ps_c = psbig.tile([P, D], F32, name="ps_c", tag="psb")
                    for dh in range(2):
                        for c in range(NKC):
                            nc.tensor.matmul(
                                ps_c[:, dh * 512 : (dh + 1) * 512],
                                attnT[r][:, c, :],
                                b16_sb[:, c, dh * 512 : (dh + 1) * 512],
                                start=(c == 0),
                                stop=(c == NKC - 1),
                            )
                    nc.scalar.mul(ctx_sb[:, r, :], ps_c[:], invS[:, r, :])
                    nc.sync.dma_start(
                        out_ext[i, r * P : (r + 1) * P, :], ctx_sb[:, r, :]
                    )

                scores_mm(0)
                softmax_half(0)
                scores_mm(1)
                attnT_half(0)
                softmax_half(1)
                if i + 1 < BPC:
                    next_h16 = emit_load_h(i + 1)
                    next_b16 = emit_load_b(i + 1)
                ctx_mm(0)
                if i + 1 < BPC:
                    next_hT = emit_hT(i + 1, next_h16)
                attnT_half(1)
                ctx_mm(1)

                if i + 1 < BPC:
                    h16_sb, b16_sb, hT_sb = next_h16, next_b16, next_hT
    _split_excess_waits(nc)
    return nc


_NC_CACHE = None


def _get_nc():
    global _NC_CACHE
    if _NC_CACHE is None:
        _NC_CACHE = build_nc()
    return _NC_CACHE


def run(b, h, W_b, trace=False):
    """Shard, execute on 8 cores, gather. Returns (ctx, BassKernelResults)."""
    assert b.shape == (B, TB, D) and h.shape == (B, TH, D)
    # inputs are consumed on-chip exclusively in fp16 -> convert on the host
    # (halves all input DMA traffic and removes every on-chip cast)
    W16 = np.ascontiguousarray(W_b[0].astype(np.float16))
    b16 = np.ascontiguousarray(b.astype(np.float16))
    h16 = np.ascontiguousarray(h.astype(np.float16))
    in_maps = []
    for c in range(N_CORES):
        sl = slice(c * BPC, (c + 1) * BPC)
        in_maps.append(
            {
                "b": b16[sl],
                "h": h16[sl],
                "w": W16,
                "ident": np.eye(P, dtype=np.float16),
            }
        )
    res = run_bass_kernel_spmd(
        _get_nc(), in_maps, core_ids=list(range(N_CORES)), trace=trace
    )
    out = np.concatenate([res.results[c]["out"] for c in range(N_CORES)], axis=0)
    return out.astype(np.float32), res


def kernel(b, h, W_b):
    out, _ = run(b, h, W_b, trace=False)
    return out
